# revision 15
# baseline (speedup 1.0000x reference)
"""Trainium2 Bass kernel for NetTGCN (gnn_message_passing).

Strategy
--------
The Chebyshev SpMMs are evaluated as dense matmuls against a densified,
normalized adjacency ("Lhat", built once on the host from edge_index as the
kernel's internal graph format).  LhatT is fp16 and row-sharded across the 8
cores (each core holds lhsT [N, N/8] = 16 MB SBUF-resident) so each core
computes its 1024 output rows per application; the full-width z (all 16
batches folded into the matmul free dim) is re-assembled with an AllGather
after every application.  The FFT (real part of a length-15 DFT) is a
constant 15x15 cosine matrix folded into W1 on the host.  Per-tap GEMMs run
on the TensorEngine via PE transposes + block-diagonal folded weights,
accumulating the conv outputs.  fc1 is contraction-sharded (each core reads
1/8 of the weight, streamed from HBM) with a final AllReduce; fc2 +
log_softmax run replicated on every core.

All floating point matmul operands are fp16 with fp32 PSUM accumulation
(host-simulated end-to-end rel err ~1.3e-3 vs the fp32 reference).
"""

import numpy as np

# ---------------------------------------------------------------- config

class CFG:
    N = 8192          # nodes
    B = 16            # batch
    T = 15            # time taps
    KCH = 25          # chebyshev order
    G1 = 32
    G2 = 64
    C = 512           # fc1 out
    D = 6             # classes
    NCORES = 8
    PHASES = 3        # 1=conv1, 2=+conv2, 3=+fc (debug bisect)
    DEBUG = False     # extra intermediate outputs

    @property
    def NLOC(self):
        return self.N // self.NCORES

    @property
    def MT(self):
        return self.NLOC // 128

    @property
    def KT(self):
        return self.N // 128

    @property
    def F1(self):
        return self.B * self.T          # conv1 z width (240)

    @property
    def PASS_B(self):
        return self.B // 2              # conv2 batches per pass (8)

    @property
    def F2(self):
        return self.PASS_B * self.G1    # conv2 z width per pass (256)


def _host_prep(cfg, x, edge_index, W1, b1, W2, b2, fc1_w, fc1_b, fc2_w, fc2_b):
    """Pure layout / format preprocessing -> per-core input maps."""
    f16 = np.float16
    N, B, T, K = cfg.N, cfg.B, cfg.T, cfg.KCH
    NC, NLOC, MT, KT = cfg.NCORES, cfg.NLOC, cfg.MT, cfg.KT

    row = np.asarray(edge_index[0], dtype=np.int64)
    col = np.asarray(edge_index[1], dtype=np.int64)
    deg = np.bincount(row, minlength=N).astype(np.float32)
    dinv = np.where(deg > 0, 1.0 / np.sqrt(np.maximum(deg, 1.0)), 0.0).astype(np.float32)
    vals = -(dinv[row] * dinv[col])
    # lhsT convention: LhatT[r, c] such that out[c] += LhatT[r, c] * z[r]
    LhatT = np.zeros((N, N), np.float32)
    np.add.at(LhatT, (row, col), vals)
    LhatT = LhatT.astype(f16)

    # x -> [p, kt, (b,t)] fp16, node n = kt*128 + p
    x_n = np.ascontiguousarray(
        np.asarray(x, np.float32).transpose(1, 0, 2).reshape(KT, 128, B * T)
        .transpose(1, 0, 2)).astype(f16)

    # fold DFT-real (cosine) matrix into W1:  xf = x @ Cf ; W1f[k] = Cf @ W1[k]
    tt = np.arange(T)
    Cf = np.cos(2 * np.pi * np.outer(tt, tt) / T).astype(np.float32)
    W1f = np.einsum('ts,ksg->ktg', Cf, np.asarray(W1, np.float32))  # [K, T, G1]

    # block-diag over quads of 4 batches: [K, 4*T, 4*G1] -> sbuf [4T, K, 4G1]
    W1blk = np.zeros((K, 4 * T, 4 * cfg.G1), np.float32)
    for b4 in range(4):
        W1blk[:, b4 * T:(b4 + 1) * T, b4 * cfg.G1:(b4 + 1) * cfg.G1] = W1f
    W1blk_sb = np.ascontiguousarray(W1blk.transpose(1, 0, 2)).astype(f16)  # [60, K, 128]

    W2blk = np.zeros((K, 4 * cfg.G1, 4 * cfg.G2), np.float32)
    for b4 in range(4):
        W2blk[:, b4 * cfg.G1:(b4 + 1) * cfg.G1, b4 * cfg.G2:(b4 + 1) * cfg.G2] = \
            np.asarray(W2, np.float32)
    W2blk_d = np.ascontiguousarray(W2blk).astype(f16)  # [K, 128, 256]

    b1row = np.tile(np.asarray(b1, np.float32), B)[None, :].astype(f16)        # [1, 512]
    b2row = np.tile(np.asarray(b2, np.float32), cfg.PASS_B)[None, :].astype(f16)  # [1, 512]
    ones_col = np.ones((1, 128), f16)
    fc1b_row = np.asarray(fc1_b, np.float32)[None, :].astype(f16)              # [1, C]
    fc2_wT = np.ascontiguousarray(
        np.asarray(fc2_w, np.float32).T.reshape(cfg.C // 128, 128, cfg.D)
        .transpose(1, 0, 2))                                                   # [128, C/128, D] f32
    fc2b_col = np.asarray(fc2_b, np.float32)[None, :]                          # [1, D]
    ones_f32 = np.ones((1, cfg.B), np.float32)

    wv = np.asarray(fc1_w, np.float32).reshape(cfg.C, N, cfg.G2)

    in_maps = []
    for c in range(NC):
        # LhatT column slice -> [p, kt, mt, m] fp16
        lt = LhatT[:, c * NLOC:(c + 1) * NLOC]
        lt = np.ascontiguousarray(
            lt.reshape(KT, 128, MT, 128).transpose(1, 0, 2, 3))
        # x local rows -> [p, mt, (b,t)]
        xl = np.asarray(x, np.float32).transpose(1, 0, 2)[c * NLOC:(c + 1) * NLOC]
        xl = np.ascontiguousarray(
            xl.reshape(MT, 128, B * T).transpose(1, 0, 2)).astype(f16)
        # fc1 weight slice -> [p, jt, cc] with jt = g*MT + mt, j = jt*128 + p
        ws = wv[:, c * NLOC:(c + 1) * NLOC, :]                    # [C, NLOC, G2]
        ws = ws.reshape(cfg.C, MT, 128, cfg.G2).transpose(2, 3, 1, 0)  # [p, g, mt, C]
        ws = np.ascontiguousarray(ws.reshape(128, cfg.G2 * MT, cfg.C)).astype(f16)
        in_maps.append(dict(
            lt=lt, x_n=x_n, x_loc=xl,
            w1blk=W1blk_sb, w2blk=W2blk_d, b1row=b1row, b2row=b2row,
            ones16=ones_col, fc1b=fc1b_row, fc2wt=fc2_wT, fc2b=fc2b_col,
            onesf32=ones_f32, wfc=ws,
        ))
    return in_maps


def _build(cfg):
    import concourse.bass as bass
    import concourse.mybir as mybir
    import concourse.tile as tile
    from concourse import bacc
    from concourse.masks import make_identity

    f16 = mybir.dt.float16
    f32 = mybir.dt.float32
    AT = mybir.ActivationFunctionType
    OP = mybir.AluOpType
    AX = mybir.AxisListType

    N, B, T, K = cfg.N, cfg.B, cfg.T, cfg.KCH
    NC, NLOC, MT, KT = cfg.NCORES, cfg.NLOC, cfg.MT, cfg.KT
    F1, F2, PASS_B = cfg.F1, cfg.F2, cfg.PASS_B
    G1, G2, C, D = cfg.G1, cfg.G2, cfg.C, cfg.D
    RG = [list(range(NC))]

    nc = bacc.Bacc("TRN2", target_bir_lowering=False, debug=False,
                   num_devices=NC)

    dt_in = {
        'lt': ([128, KT, MT, 128], f16),
        'x_n': ([128, KT, F1], f16),
        'x_loc': ([128, MT, F1], f16),
        'w1blk': ([4 * T, K, 4 * G1], f16),
        'w2blk': ([K, 4 * G1, 4 * G2], f16),
        'b1row': ([1, B * G1], f16),
        'b2row': ([1, PASS_B * G2], f16),
        'ones16': ([1, 128], f16),
        'fc1b': ([1, C], f16),
        'fc2wt': ([128, C // 128, D], f32),
        'fc2b': ([1, D], f32),
        'onesf32': ([1, B], f32),
        'wfc': ([128, G2 * MT, C], f16),
    }
    din = {k: nc.dram_tensor(k, shp, dt, kind="ExternalInput").ap()
           for k, (shp, dt) in dt_in.items()}
    dout = nc.dram_tensor("out", [B, D], f32, kind="ExternalOutput").ap()
    if cfg.DEBUG:
        dbg_h1 = nc.dram_tensor("dbg_h1", [128, MT, B * G1], f16,
                                kind="ExternalOutput").ap()
        dbg_h2 = nc.dram_tensor("dbg_h2", [128, MT, B, G2], f16,
                                kind="ExternalOutput").ap()
        dbg_z1 = nc.dram_tensor("dbg_z1", [128, MT, F1], f16,
                                kind="ExternalOutput").ap()
        dbg_acc = nc.dram_tensor("dbg_acc", [128, MT, B * G1], f32,
                                 kind="ExternalOutput").ap()
        dbg_zg = nc.dram_tensor("dbg_zg", [128, KT, F1], f16,
                                kind="ExternalOutput").ap()

    with tile.TileContext(nc) as tc:
        with (
            tc.tile_pool(name="const", bufs=1) as constp,
            tc.tile_pool(name="dram", bufs=1, space="DRAM") as dramp,
        ):
            # ---------------- constants / persistent state
            LT = constp.tile([128, KT, MT, 128], f16)
            nc.sync.dma_start(LT[:], din['lt'])
            ident16 = constp.tile([128, 128], f16)
            make_identity(nc, ident16[:])
            identf32 = constp.tile([32, 32], f32)
            make_identity(nc, identf32[:])
            ones16 = constp.tile([1, 128], f16)
            nc.sync.dma_start(ones16[:], din['ones16'])

            h1_dram = dramp.tile([KT, 128, B * G1], f16)     # gathered h1
            # phase-scoped persistent pools (closed explicitly at the end)
            p1 = tc.tile_pool(name="p1", bufs=1)
            p1p = p1.__enter__()
            h1loc = p1p.tile([128, MT, B * G1], f16)         # own rows of h1

            def kgemm(tloc_ap_fn, acc, pgpool, trpool, trsb_pool, wblk_ap,
                      brow, n_quads, twidth, owidth, add_bias, mt):
                """Per-tap GEMM for one M-tile: acc[:, mt, :] +=
                transpose(T_k local quads) @ Wblk (block-diag over 4 batches).
                wblk_ap: [twidth, owidth] AP, same for every quad."""
                pg = pgpool.tile([128, n_quads * owidth], f32, tag="pg")
                for q in range(n_quads):
                    tp = trpool.tile([128, 128], f16, tag="tp")
                    src = tloc_ap_fn(mt, q)                   # [128, twidth]
                    nc.tensor.transpose(tp[:twidth, :], src, ident16[:])
                    tsb = trsb_pool.tile([128, 128], f16, tag="tsb")
                    nc.vector.tensor_copy(tsb[:twidth, :], tp[:twidth, :])
                    nc.tensor.matmul(
                        pg[:, q * owidth:(q + 1) * owidth],
                        tsb[:twidth, :], wblk_ap,
                        start=True, stop=not add_bias,
                    )
                    if add_bias:
                        nc.tensor.matmul(
                            pg[:, q * owidth:(q + 1) * owidth],
                            ones16[:1, :128],
                            brow[:1, q * owidth:(q + 1) * owidth],
                            start=False, stop=True,
                        )
                nc.vector.tensor_tensor(acc[:, mt, :], acc[:, mt, :], pg[:],
                                        OP.add)

            # ================ conv1 =================
            with (
                tc.tile_pool(name="c1", bufs=1) as c1p,
                tc.tile_pool(name="znl", bufs=1) as znlp,
                tc.tile_pool(name="psz", bufs=2, space="PSUM") as pszp,
                tc.tile_pool(name="pst", bufs=2, space="PSUM") as pstp,
                tc.tile_pool(name="psg", bufs=2, space="PSUM") as psgp,
                tc.tile_pool(name="trsb", bufs=3) as trsbp,
            ):
                w1blk = c1p.tile([4 * T, K, 4 * G1], f16)
                nc.sync.dma_start(w1blk[:], din['w1blk'])
                b1row = c1p.tile([1, B * G1], f16)
                nc.sync.dma_start(b1row[:], din['b1row'])
                z = c1p.tile([128, KT, F1], f16)
                nc.sync.dma_start(z[:], din['x_n'])
                xloc = c1p.tile([128, MT, F1], f16)
                nc.sync.dma_start(xloc[:], din['x_loc'])
                acc1 = c1p.tile([128, MT, B * G1], f32)
                nc.vector.memset(acc1[:], 0.0)
                znl = [znlp.tile([128, MT, F1], f16, tag=f"znl{i}",
                                 name=f"znl{i}") for i in range(2)]
                gin1 = dramp.tile([MT, 128, F1], f16)
                gout1 = dramp.tile([KT, 128, F1], f16)

                # k = 0 term (T_0 = z0, local slice = xloc)
                for mt in range(MT):
                    kgemm(lambda m, q: xloc[:, m, q * 60:(q + 1) * 60],
                          acc1, psgp, pstp, trsbp, w1blk[:, 0, :], b1row,
                          n_quads=4, twidth=60, owidth=128, add_bias=False,
                          mt=mt)

                nc.vector.tensor_copy(znl[0][:], xloc[:])
                for kk in range(1, K):
                    cur = znl[kk % 2]
                    for mt in range(MT):
                        ps = pszp.tile([128, F1], f32, tag="psz")
                        for kt in range(KT):
                            nc.tensor.matmul(ps[:], LT[:, kt, mt, :],
                                             z[:, kt, :],
                                             start=(kt == 0),
                                             stop=(kt == KT - 1))
                        if kk == 1:
                            nc.vector.tensor_copy(cur[:, mt, :], ps[:])
                        else:
                            # T_k = 2 Lhat T_{k-1} - T_{k-2}; T_{k-2} lives
                            # in-place in this parity buffer
                            nc.vector.scalar_tensor_tensor(
                                cur[:, mt, :], ps[:], 2.0, cur[:, mt, :],
                                OP.mult, OP.subtract)
                        if cfg.DEBUG and kk == 1:
                            nc.sync.dma_start(dbg_z1[:, mt, :], cur[:, mt, :])
                        kgemm(lambda m, q: cur[:, m, q * 60:(q + 1) * 60],
                              acc1, psgp, pstp, trsbp, w1blk[:, kk, :], b1row,
                              n_quads=4, twidth=60, owidth=128,
                              add_bias=(kk == K - 1), mt=mt)
                    # all-gather cur -> z
                    if kk < K - 1:
                        nc.sync.dma_start(
                            gin1[:].rearrange("m p f -> p m f"), cur[:])
                        nc.gpsimd.collective_compute(
                            "AllGather", OP.bypass, replica_groups=RG,
                            ins=[gin1[:]], outs=[gout1[:]])
                        nc.sync.dma_start(
                            z[:], gout1[:].rearrange("k p f -> p k f"))
                        if cfg.DEBUG and kk == 1:
                            nc.sync.dma_start(dbg_zg, z[:])

                # h1 = relu(acc1 + b1) (bias already added), gather to DRAM
                nc.vector.tensor_scalar_max(h1loc[:], acc1[:], 0.0)
                if cfg.DEBUG:
                    nc.sync.dma_start(dbg_acc[:], acc1[:])
                    nc.sync.dma_start(dbg_h1[:], h1loc[:])
                gh1 = dramp.tile([MT, 128, B * G1], f16)
                nc.sync.dma_start(gh1[:].rearrange("m p f -> p m f"), h1loc[:])
                nc.gpsimd.collective_compute(
                    "AllGather", OP.bypass, replica_groups=RG,
                    ins=[gh1[:]], outs=[h1_dram[:]])

            # ================ conv2 (2 passes of 8 batches) =================
            p2 = tc.tile_pool(name="p2", bufs=1)
            p2p = p2.__enter__()
            h2conv = p2p.tile([128, MT, B, G2], f16)         # fc1 lhsT source
            if cfg.PHASES < 2:
                zz = constp.tile([B, D], f32)
                nc.vector.memset(zz[:], 0.0)
                nc.sync.dma_start(dout, zz[:])
                p2.__exit__(None, None, None)
                p1.__exit__(None, None, None)
                return nc
            with (
                tc.tile_pool(name="c2", bufs=1) as c2p,
                tc.tile_pool(name="znl2", bufs=1) as znl2p,
                tc.tile_pool(name="w2s", bufs=2) as w2sp,
                tc.tile_pool(name="psz2", bufs=2, space="PSUM") as psz2p,
                tc.tile_pool(name="pst2", bufs=2, space="PSUM") as pst2p,
                tc.tile_pool(name="psg2", bufs=2, space="PSUM") as psg2p,
                tc.tile_pool(name="trsb2", bufs=3) as trsb2p,
            ):
                b2row = c2p.tile([1, PASS_B * G2], f16)
                nc.sync.dma_start(b2row[:], din['b2row'])
                z2 = c2p.tile([128, KT, F2], f16)
                acc2 = c2p.tile([128, MT, PASS_B * G2], f16)
                znl2 = [znl2p.tile([128, MT, F2], f16, tag=f"znl2{i}",
                                   name=f"znl2{i}") for i in range(2)]
                gin2 = dramp.tile([MT, 128, F2], f16)
                gout2 = dramp.tile([KT, 128, F2], f16)

                for pb in range(2):
                    fs = pb * F2
                    nc.sync.dma_start(
                        z2[:], h1_dram[:, :, fs:fs + F2]
                        .rearrange("k p f -> p k f"))
                    nc.vector.memset(acc2[:], 0.0)

                    # k = 0 term (T_0 local slice = h1loc columns of this pass)
                    w2k0 = w2sp.tile([4 * G1, 4 * G2], f16, tag="w2k")
                    nc.sync.dma_start(w2k0[:], din['w2blk'][0])
                    for mt in range(MT):
                        kgemm(lambda m, q: h1loc[:, m, fs + q * 128:fs + (q + 1) * 128],
                              acc2, psg2p, pst2p, trsb2p, w2k0[:], b2row,
                              n_quads=2, twidth=128, owidth=256,
                              add_bias=False, mt=mt)

                    nc.vector.tensor_copy(znl2[0][:],
                                          h1loc[:, :, fs:fs + F2])
                    for kk in range(1, K):
                        cur = znl2[kk % 2]
                        w2k = w2sp.tile([4 * G1, 4 * G2], f16, tag="w2k")
                        nc.sync.dma_start(w2k[:], din['w2blk'][kk])
                        for mt in range(MT):
                            ps = psz2p.tile([128, F2], f32, tag="psz2")
                            for kt in range(KT):
                                nc.tensor.matmul(ps[:], LT[:, kt, mt, :],
                                                 z2[:, kt, :],
                                                 start=(kt == 0),
                                                 stop=(kt == KT - 1))
                            if kk == 1:
                                nc.vector.tensor_copy(cur[:, mt, :], ps[:])
                            else:
                                nc.vector.scalar_tensor_tensor(
                                    cur[:, mt, :], ps[:], 2.0,
                                    cur[:, mt, :], OP.mult, OP.subtract)
                            kgemm(lambda m, q: cur[:, m, q * 128:(q + 1) * 128],
                                  acc2, psg2p, pst2p, trsb2p, w2k[:], b2row,
                                  n_quads=2, twidth=128, owidth=256,
                                  add_bias=(kk == K - 1), mt=mt)
                        if kk < K - 1:
                            nc.sync.dma_start(
                                gin2[:].rearrange("m p f -> p m f"), cur[:])
                            nc.gpsimd.collective_compute(
                                "AllGather", OP.bypass, replica_groups=RG,
                                ins=[gin2[:]], outs=[gout2[:]])
                            nc.sync.dma_start(
                                z2[:], gout2[:].rearrange("k p f -> p k f"))

                    # h2 = relu(acc2 + b2) into h2conv batch half
                    nc.vector.tensor_scalar_max(
                        h2conv[:, :, pb * PASS_B:(pb + 1) * PASS_B, :],
                        acc2[:].rearrange("p m (b g) -> p m b g", b=PASS_B),
                        0.0)

            if cfg.DEBUG:
                nc.sync.dma_start(dbg_h2[:], h2conv[:])

            # ================ fc1 (streamed weights) =================
            if cfg.PHASES < 3:
                zz = constp.tile([B, D], f32)
                nc.vector.memset(zz[:], 0.0)
                nc.sync.dma_start(dout, zz[:])
                p2.__exit__(None, None, None)
                p1.__exit__(None, None, None)
                return nc
            with (
                tc.tile_pool(name="fcw", bufs=2) as fcwp,
                tc.tile_pool(name="fcps", bufs=1, space="PSUM") as fcpsp,
                tc.tile_pool(name="fcsb", bufs=1) as fcsbp,
                tc.tile_pool(name="fcps2", bufs=2, space="PSUM") as fcps2p,
            ):
                JT = G2 * MT            # 512 j-tiles
                JBLK = 8
                psfc = fcpsp.tile([B, C], f32)
                fc1b_sb = fcsbp.tile([1, C], f16)
                nc.sync.dma_start(fc1b_sb[:], din['fc1b'])
                for jb in range(JT // JBLK):
                    wbuf = fcwp.tile([128, JBLK, C], f16, tag="wbuf")
                    nc.sync.dma_start(wbuf[:],
                                      din['wfc'][:, jb * JBLK:(jb + 1) * JBLK, :])
                    for ji in range(JBLK):
                        jt = jb * JBLK + ji
                        g, mt = jt // MT, jt % MT
                        nc.tensor.matmul(psfc[:], h2conv[:, mt, :, g],
                                         wbuf[:, ji, :],
                                         start=(jt == 0), stop=False)
                # bias via ones trick (last accumulation closes the group)
                ones16_f16 = ones16
                nc.tensor.matmul(psfc[:], ones16_f16[:1, :B], fc1b_sb[:1, :],
                                 start=False, stop=True)

                # transpose [B, C] -> [C/128 tiles of [128, B]]
                hsb = fcsbp.tile([B, C], f32)
                nc.vector.tensor_copy(hsb[:], psfc[:])
                hT = fcsbp.tile([128, C // 128, B], f32)
                for t4 in range(C // 128):
                    tp = fcps2p.tile([128, B], f32, tag="fct")
                    nc.tensor.transpose(tp[:], hsb[:, t4 * 128:(t4 + 1) * 128],
                                        identf32[:B, :B])
                    nc.vector.tensor_copy(hT[:, t4, :], tp[:])

                arin = dramp.tile([128, C // 128, B], f32)
                arout = dramp.tile([128, C // 128, B], f32)
                nc.sync.dma_start(arin[:], hT[:])
                nc.gpsimd.collective_compute(
                    "AllReduce", OP.add, replica_groups=RG,
                    ins=[arin[:]], outs=[arout[:]])
                hTr = fcsbp.tile([128, C // 128, B], f32)
                nc.sync.dma_start(hTr[:], arout[:])

                # fc2: out[d, b] = fc2_w[d, :] @ h[:, b]
                fc2wt = fcsbp.tile([128, C // 128, D], f32)
                nc.sync.dma_start(fc2wt[:], din['fc2wt'])
                fc2b = fcsbp.tile([1, D], f32)
                nc.sync.dma_start(fc2b[:], din['fc2b'])
                onesf32 = fcsbp.tile([1, B], f32)
                nc.sync.dma_start(onesf32[:], din['onesf32'])
                ps2 = fcps2p.tile([D, B], f32, tag="ps2")
                for kt in range(C // 128):
                    nc.tensor.matmul(ps2[:], fc2wt[:, kt, :], hTr[:, kt, :],
                                     start=(kt == 0), stop=False)
                nc.tensor.matmul(ps2[:], fc2b[:1, :], onesf32[:1, :],
                                 start=False, stop=True)

                s2 = fcsbp.tile([D, B], f32)
                nc.vector.tensor_copy(s2[:], ps2[:])
                ps3 = fcps2p.tile([B, D], f32, tag="ps3")
                nc.tensor.transpose(ps3[:], s2[:], identf32[:D, :D])
                sm = fcsbp.tile([B, D], f32)
                nc.vector.tensor_copy(sm[:], ps3[:])

                # log_softmax over D (free axis)
                mx = fcsbp.tile([B, 1], f32)
                nc.vector.tensor_reduce(mx[:], sm[:], AX.X, OP.max)
                xm = fcsbp.tile([B, D], f32)
                nc.vector.tensor_single_scalar(xm[:], sm[:], mx[:], OP.subtract)
                ex = fcsbp.tile([B, D], f32)
                nc.scalar.activation(ex[:], xm[:], AT.Exp)
                sume = fcsbp.tile([B, 1], f32)
                nc.vector.tensor_reduce(sume[:], ex[:], AX.X, OP.add)
                lse = fcsbp.tile([B, 1], f32)
                nc.scalar.activation(lse[:], sume[:], AT.Ln)
                res = fcsbp.tile([B, D], f32)
                nc.vector.tensor_single_scalar(res[:], xm[:], lse[:],
                                               OP.subtract)
                nc.sync.dma_start(dout, res[:])
            p2.__exit__(None, None, None)
            p1.__exit__(None, None, None)

    return nc


def _run(cfg, inputs, trace=False):
    in_maps = _host_prep(cfg, **inputs)
    nc = _build(cfg)
    nc.compile()
    from concourse import bass_utils
    res = bass_utils.run_bass_kernel_spmd(
        nc, in_maps, core_ids=list(range(cfg.NCORES)), trace=trace)
    return np.asarray(res.results[0]['out'], np.float32).copy(), res


def kernel(**inputs):
    out, _ = _run(CFG(), inputs)
    return out
